# revision 8
# baseline (speedup 1.0000x reference)
"""TRN2 Bass kernel for CrossAttention (B=16, L=1024, H=A=1024, fp32).

Strategy (8 NeuronCores, data-parallel over batch, 2 batch elements/core),
with algebraic fusion to avoid weight transposes and one projection:

  scores = (meme Wq^T + bq)(text Wk^T + bk)^T ; softmax over k ; @ (emoji Wv^T + bv)

  1. bk shifts every softmax row by a constant -> drops out exactly.
  2. Mt[h2,h] = sum_a Wq[a,h2] Wk[a,h] is computed ONCE from both weights in
     natural layout (contraction over a = partition dim).  Then per batch:
        G[h,q]  = sum_h2 Mt[h2,h] meme^T[h2,q] + c[h]   (c = Wk^T bq)
        S^T[k,q] = sum_h text^T[h,k] G[h,q]             == Q K0^T transposed
  3. softmax skips max-subtraction (logits bounded ~83; exp fits fp32/bf16),
     E^T = exp(S^T) in bf16 straight out of PSUM on the Scalar engine.
  4. V-projection is fused into the output:  O = (E/s) emoji Wv^T + bv:
        T^T[h,q] = sum_k emoji[k,h] E^T[k,q]   (emoji natural, bf16)
        O[q,a]   = sum_h T^T[h,q] WvT[h,a]     (WvT transposed once, bf16)
        row sums s[q] via N=1 matmuls vs a ones vector; final scale+bias
        on the PSUM->SBUF copy (ACT scale=1/s, DVE +bv).

  All feature/weight DMA is row-granular [128,1024] (4KB per-partition
  lines).  textT blocks are produced just-in-time per k-tile and freed
  after use.  PSUM drains are split between Scalar (ACT) and Vector (DVE)
  engines so the PE never waits long on a single drain engine.  Mt is
  accumulated ac-outer in rounds so it consumes weight chunks as the DMA
  delivers them.
"""

import sys

sys.path.insert(0, "/opt/trn_rl_repo")

import contextlib
import numpy as np
import concourse.bacc as bacc
import concourse.bass as bass
import concourse.mybir as mybir
from concourse.tile import TileContext
from concourse.bass_utils import run_bass_kernel_spmd
from concourse.masks import make_identity

F32 = mybir.dt.float32
F32R = mybir.dt.float32r
BF16 = mybir.dt.bfloat16
EXP = mybir.ActivationFunctionType.Exp
COPY = mybir.ActivationFunctionType.Copy
IDENT = mybir.ActivationFunctionType.Identity

P = 128
B, L, H, A = 16, 1024, 1024, 1024
NCORES = 8
NB = B // NCORES  # batch elements per core
NH = H // P       # 8 chunks


def _build_program(repeat=1):
    nc = bacc.Bacc("TRN2", target_bir_lowering=False, debug=False, num_devices=NCORES)

    xm = nc.declare_dram_parameter("xm", [NB, L, H], F32R, isOutput=False)
    xt_ = nc.declare_dram_parameter("xt", [NB, L, H], F32R, isOutput=False)
    xe = nc.declare_dram_parameter("xe", [NB, L, H], F32, isOutput=False)
    wq = nc.declare_dram_parameter("wq", [A, H], F32R, isOutput=False)
    wk = nc.declare_dram_parameter("wk", [A, H], F32R, isOutput=False)
    wv = nc.declare_dram_parameter("wv", [A, H], F32R, isOutput=False)
    bq = nc.declare_dram_parameter("bq", [A], F32R, isOutput=False)
    bk = nc.declare_dram_parameter("bk", [A], F32, isOutput=False)  # unused
    bv = nc.declare_dram_parameter("bv", [A], F32, isOutput=False)
    o = nc.declare_dram_parameter("o", [NB, L, A], F32, isOutput=True)

    with TileContext(nc) as tc:
        with (
            tc.tile_pool(name="sgl", bufs=1) as sgl,
            tc.tile_pool(name="big1", bufs=1) as big1,
            tc.tile_pool(name="sm", bufs=4) as smp,
            tc.tile_pool(name="stgr", bufs=5) as stgr,
            tc.tile_pool(name="stge", bufs=1) as stge,
            tc.tile_pool(name="ps", bufs=6, space="PSUM") as psp,
            tc.tile_pool(name="ps2", bufs=2, space="PSUM") as ps2,
        ):
            rep_ctx = tc.For_i(0, repeat, 1) if repeat > 1 else contextlib.nullcontext()
            with rep_ctx:
                ident_f = sgl.tile([P, P], F32, tag="ident_f")
                make_identity(nc, ident_f)
                ident = sgl.tile([P, P], F32R, tag="ident")
                nc.vector.tensor_copy(ident, ident_f)
                ones_bf = sgl.tile([P, 1], BF16, tag="ones_bf")
                nc.vector.memset(ones_bf, 1.0)
                cT = sgl.tile([P, NH], F32, tag="cT")

                # persistent big tiles
                Mt = big1.tile([P, NH, H], F32R, tag="mt")        # [p, h2, h]
                WvT = big1.tile([P, NH, A], BF16, tag="wvt")      # [p, hc, a]
                memeT = big1.tile([P, NH, L], F32R, tag="memeT")  # [p, hc, q]

                def transp_rows(row_t, dst):
                    """row_t [128,1024]: 8 [128,128]-block transposes -> 2 psum
                    tiles -> dst[:, 0:4, :] and dst[:, 4:8, :] (contiguous)."""
                    for g4 in range(2):
                        pst = psp.tile([P, 4, P], F32R, tag="mm", name="pst")
                        for j in range(4):
                            hc = g4 * 4 + j
                            nc.tensor.transpose(
                                pst[:, j, :], row_t[:, hc * P : (hc + 1) * P], ident
                            )
                        nc.vector.tensor_copy(
                            dst[:, g4 * 4 : (g4 + 1) * 4, :], pst
                        )

                def transp_feature(rows, dst):
                    """rows: 8 [128,1024] row tiles; hc-outer transposes so each
                    psum packs 4 q-blocks of one h-chunk -> contiguous drains."""
                    for g in range(2):
                        for hc in range(NH):
                            pst = psp.tile([P, 512], F32R, tag="mm", name="pst")
                            for j in range(4):
                                nc.tensor.transpose(
                                    pst[:, j * P : (j + 1) * P],
                                    rows[g * 4 + j][:, hc * P : (hc + 1) * P],
                                    ident,
                                )
                            dv = dst[:, hc, g * 512 : (g + 1) * 512]
                            if hc % 2 == 0:
                                nc.vector.tensor_copy(dv, pst)
                            else:
                                nc.scalar.activation(dv, pst, COPY)

                def meme_rows(b):
                    rows = []
                    for rt in range(NH):
                        row = stgr.tile([P, H], F32R, tag="stgr", name=f"mrow{b}_{rt}")
                        nc.sync.dma_start(
                            out=row, in_=xm.ap()[b, rt * P : (rt + 1) * P, :]
                        )
                        rows.append(row)
                    return rows

                def load_memeT(b):
                    transp_feature(meme_rows(b), memeT)

                # ---- setup: first weight chunk, meme(b0) transposes, Mt, c, WvT
                with tc.tile_pool(name="wn", bufs=16) as wnp:
                    wqn = []
                    wkn = []

                    def wload(ci):
                        tq = wnp.tile([P, H], F32R, tag="wn", name=f"wqn{ci}")
                        nc.sync.dma_start(out=tq, in_=wq.ap()[ci * P : (ci + 1) * P, :])
                        wqn.append(tq)
                        tk = wnp.tile([P, H], F32R, tag="wn", name=f"wkn{ci}")
                        nc.sync.dma_start(out=tk, in_=wk.ap()[ci * P : (ci + 1) * P, :])
                        wkn.append(tk)

                    for ci in range(NH):
                        wload(ci)
                    bvb = sgl.tile([P, A], F32, tag="bvb")
                    nc.sync.dma_start(out=bvb, in_=bv.ap().partition_broadcast(P))
                    bqc = sgl.tile([P, NH + 1], F32R, tag="bqc")
                    zrow = sgl.tile([P, 1], F32, tag="zrow")
                    nc.vector.memset(zrow, 0.0)
                    nc.vector.tensor_copy(bqc[:, NH : NH + 1], zrow)
                    nc.sync.dma_start(
                        out=bqc[:, 0:NH], in_=bq.ap().rearrange("(c p) -> p c", p=P)
                    )
                    b0_rows = meme_rows(0)

                    # Mt[h2][:, h] = sum_a Wq[a,h2] Wk[a,h]; ac-outer rounds of 3
                    for rnd, nh2 in ((0, 3), (3, 3), (6, 2)):
                        psts = [
                            psp.tile([P, 512], F32, tag="mm", name=f"psmt{i}")
                            for i in range(2 * nh2)
                        ]
                        for ac in range(NH):
                            for h2r in range(nh2):
                                h2 = rnd + h2r
                                for g in range(2):
                                    nc.tensor.matmul(
                                        psts[h2r * 2 + g],
                                        lhsT=wqn[ac][:, h2 * P : (h2 + 1) * P],
                                        rhs=wkn[ac][:, g * 512 : (g + 1) * 512],
                                        start=(ac == 0),
                                        stop=(ac == NH - 1),
                                    )
                        for h2r in range(nh2):
                            h2 = rnd + h2r
                            nc.vector.tensor_copy(
                                Mt[:, h2, 0:512], psts[h2r * 2]
                            )
                            nc.vector.tensor_copy(
                                Mt[:, h2, 512:1024], psts[h2r * 2 + 1]
                            )
                    # c[h_tile] = sum_a Wk[a, h] bq[a]
                    for ht in range(NH):
                        psc = ps2.tile([P, 2], F32, tag="sum")
                        for ac in range(NH):
                            nc.tensor.matmul(
                                psc,
                                lhsT=wkn[ac][:, ht * P : (ht + 1) * P],
                                rhs=bqc[:, ac : ac + 2],
                                start=(ac == 0),
                                stop=(ac == NH - 1),
                            )
                        nc.vector.tensor_copy(cT[:, ht : ht + 1], psc[:, 0:1])

                    transp_feature(b0_rows, memeT)

                def load_wvT():
                    rows = []
                    for at in range(NH):
                        row = stgr.tile([P, H], F32R, tag="stgr", name=f"vrow{at}")
                        nc.sync.dma_start(
                            out=row, in_=wv.ap()[at * P : (at + 1) * P, :]
                        )
                        rows.append(row)
                    transp_feature(rows, WvT)

                # ---- main loop pools + per-batch work ----
                with (
                    tc.tile_pool(name="big2", bufs=1) as big2,
                    tc.tile_pool(name="tb", bufs=2) as tbp,
                    tc.tile_pool(name="et", bufs=2) as etp,
                    tc.tile_pool(name="tt", bufs=2) as ttp,
                    tc.tile_pool(name="op", bufs=2) as opp,
                ):
                    G = big2.tile([P, NH, L], F32R, tag="g")    # [p, hc, q]
                    EM = big2.tile([P, NH, H], BF16, tag="em")  # [p, kc, h]

                    def load_emoji(b):
                        for kc in range(NH):
                            row = stge.tile(
                                [P, H], F32, tag="stge", name=f"erow{b}_{kc}"
                            )
                            nc.sync.dma_start(
                                out=row, in_=xe.ap()[b, kc * P : (kc + 1) * P, :]
                            )
                            nc.scalar.activation(EM[:, kc, :], row, COPY)

                    for b in range(NB):
                        if b > 0:
                            load_memeT(b)

                        # G[h, q] = sum_h2 Mt[h2, h] meme^T[h2, q] + c[h]
                        for ht in range(NH):
                            psts = [
                                psp.tile([P, 512], F32, tag="mm", name=f"ps{i}")
                                for i in range(2)
                            ]
                            for h2 in range(NH):
                                for qb in range(2):
                                    nc.tensor.matmul(
                                        psts[qb],
                                        lhsT=Mt[:, h2, ht * P : (ht + 1) * P],
                                        rhs=memeT[:, h2, qb * 512 : (qb + 1) * 512],
                                        start=(h2 == 0),
                                        stop=(h2 == NH - 1),
                                    )
                            for qb in range(2):
                                nc.scalar.activation(
                                    G[:, ht, qb * 512 : (qb + 1) * 512],
                                    psts[qb],
                                    IDENT,
                                    bias=cT[:, ht : ht + 1],
                                )

                        if b == 0:
                            load_wvT()

                        # S^T per k-tile with streamed text^T blocks; exp -> E^T
                        Et = [
                            etp.tile([P, NH, 512], BF16, tag="et", name=f"et{b}_{q}")
                            for q in range(2)
                        ]
                        for kt in range(NH):
                            row = stgr.tile(
                                [P, H], F32R, tag="stgr", name=f"trow{b}_{kt}"
                            )
                            nc.sync.dma_start(
                                out=row, in_=xt_.ap()[b, kt * P : (kt + 1) * P, :]
                            )
                            tb = tbp.tile([P, NH, P], F32R, tag="tb", name="tb")
                            transp_rows(row, tb)
                            psts = [
                                psp.tile([P, 512], F32, tag="mm", name=f"ps{i}")
                                for i in range(2)
                            ]
                            for hc in range(NH):
                                for qb in range(2):
                                    nc.tensor.matmul(
                                        psts[qb],
                                        lhsT=tb[:, hc, :],
                                        rhs=G[:, hc, qb * 512 : (qb + 1) * 512],
                                        start=(hc == 0),
                                        stop=(hc == NH - 1),
                                    )
                            for qb in range(2):
                                nc.scalar.activation(Et[qb][:, kt, :], psts[qb], EXP)

                        load_emoji(b)

                        # T^T[h, q] = sum_k emoji[k, h] E^T[k, q]
                        Tt = [
                            ttp.tile([P, NH, 512], BF16, tag="tt", name=f"tt{b}_{q}")
                            for q in range(2)
                        ]
                        for ht in range(NH):
                            psts = [
                                psp.tile([P, 512], F32, tag="mm", name=f"ps{i}")
                                for i in range(2)
                            ]
                            for kc in range(NH):
                                for qb in range(2):
                                    nc.tensor.matmul(
                                        psts[qb],
                                        lhsT=EM[:, kc, ht * P : (ht + 1) * P],
                                        rhs=Et[qb][:, kc, :],
                                        start=(kc == 0),
                                        stop=(kc == NH - 1),
                                    )
                            nc.vector.tensor_copy(Tt[0][:, ht, :], psts[0])
                            nc.scalar.activation(Tt[1][:, ht, :], psts[1], COPY)

                        # O[q_tile, :] = (sum_h T^T[h,q] WvT[h,a]) / s[q] + bv
                        for qb in range(2):
                            for qt in range(4):
                                qs = qt * P
                                ps0 = psp.tile([P, 512], F32, tag="mm")
                                ps1 = psp.tile([P, 512], F32, tag="mm")
                                pss = ps2.tile([P, 1], F32, tag="sum")
                                for kc in range(NH):
                                    nc.tensor.matmul(
                                        pss,
                                        lhsT=Et[qb][:, kc, qs : qs + P],
                                        rhs=ones_bf,
                                        start=(kc == 0),
                                        stop=(kc == NH - 1),
                                    )
                                for hc in range(NH):
                                    st, sp = (hc == 0), (hc == NH - 1)
                                    nc.tensor.matmul(
                                        ps0,
                                        lhsT=Tt[qb][:, hc, qs : qs + P],
                                        rhs=WvT[:, hc, 0:512],
                                        start=st,
                                        stop=sp,
                                    )
                                    nc.tensor.matmul(
                                        ps1,
                                        lhsT=Tt[qb][:, hc, qs : qs + P],
                                        rhs=WvT[:, hc, 512:1024],
                                        start=st,
                                        stop=sp,
                                    )
                                rec = smp.tile([P, 1], F32, tag="rec")
                                nc.vector.reciprocal(rec, pss)
                                o_t = opp.tile([P, A], F32, tag="op")
                                q0 = qb * 512 + qs
                                for hf, psh in ((0, ps0), (1, ps1)):
                                    sl = slice(hf * 512, (hf + 1) * 512)
                                    nc.scalar.activation(
                                        o_t[:, sl], psh, COPY, scale=rec
                                    )
                                    nc.vector.tensor_add(
                                        o_t[:, sl], o_t[:, sl], bvb[:, sl]
                                    )
                                    nc.scalar.dma_start(
                                        out=o.ap()[b, q0 : q0 + P, sl], in_=o_t[:, sl]
                                    )

    nc.compile()
    return nc


_NC = {}


def _get_nc(repeat=1):
    if repeat not in _NC:
        _NC[repeat] = _build_program(repeat)
    return _NC[repeat]


def _run(inputs, trace=False, repeat=1):
    nc = _get_nc(repeat)
    c = np.ascontiguousarray

    def f32c(x):
        return c(np.asarray(x, dtype=np.float32))

    meme = f32c(inputs["meme_features"])
    text = f32c(inputs["text_features"])
    emoji = f32c(inputs["emoji_features"])
    full = {
        "wq": f32c(inputs["Wq"]),
        "wk": f32c(inputs["Wk"]),
        "wv": f32c(inputs["Wv"]),
        "bq": f32c(inputs["bq"]),
        "bk": f32c(inputs["bk"]),
        "bv": f32c(inputs["bv"]),
    }
    in_maps = []
    for i in range(NCORES):
        s = slice(i * NB, (i + 1) * NB)
        in_maps.append(
            {"xm": c(meme[s]), "xt": c(text[s]), "xe": c(emoji[s]), **full}
        )
    res = run_bass_kernel_spmd(nc, in_maps, list(range(NCORES)), trace=trace)
    out = np.concatenate([res.results[i]["o"] for i in range(NCORES)], axis=0)
    return out, res


def kernel(**inputs):
    out, _ = _run(inputs, trace=False)
    return out


if __name__ == "__main__":
    rng = np.random.default_rng(0)
    s = 1.0 / np.sqrt(H)
    inputs = {
        "meme_features": rng.standard_normal((B, L, H), dtype=np.float32),
        "text_features": rng.standard_normal((B, L, H), dtype=np.float32),
        "emoji_features": rng.standard_normal((B, L, H), dtype=np.float32),
        "Wq": rng.uniform(-s, s, (A, H)).astype(np.float32),
        "bq": rng.uniform(-s, s, A).astype(np.float32),
        "Wk": rng.uniform(-s, s, (A, H)).astype(np.float32),
        "bk": rng.uniform(-s, s, A).astype(np.float32),
        "Wv": rng.uniform(-s, s, (A, H)).astype(np.float32),
        "bv": rng.uniform(-s, s, A).astype(np.float32),
    }
    out = kernel(**inputs)
    q = np.einsum("blh,ah->bla", inputs["meme_features"], inputs["Wq"]) + inputs["bq"]
    k = np.einsum("blh,ah->bla", inputs["text_features"], inputs["Wk"]) + inputs["bk"]
    v = np.einsum("blh,ah->bla", inputs["emoji_features"], inputs["Wv"]) + inputs["bv"]
    sc = np.einsum("bqa,bka->bqk", q, k)
    sc -= sc.max(-1, keepdims=True)
    w = np.exp(sc)
    w /= w.sum(-1, keepdims=True)
    ref = np.einsum("bqk,bka->bqa", w, v)
    err = np.linalg.norm(out - ref) / np.linalg.norm(ref)
    print(f"smoke rel err: {err:.3e}")
